# revision 23
# baseline (speedup 1.0000x reference)
"""Multi-head causal attention (B=4, S=2048, D=2048, H=16) on 8 trn2 cores.

Sharding: core c handles batch b = c//2 and head-group g = c%2 (8 heads).
Each core computes q/k/v projections for its heads, causal attention, and a
partial out_proj over its dv-slice. Host sums the two partials per batch.

v8 design (fused, bf16, engine-balanced softmax-Z):
  - All tensors stored bf16 in SBUF; matmul accumulation in f32 PSUM.
  - Single PSUM pool with two tags ('sc','acc', each [128,1024] f32 = 2
    banks, bufs=2 -> 8 banks) shared by ALL phases: no pool-close barriers
    between phases (the 1a->1b->2->3 transitions stay back-to-back on PE).
  - Phase 1a: qkT[e, s] per ec into two [128,1024] PSUM tiles.
  - Phase 1b: v[s, ev] (exp(alibi) folded into v during PSUM evacuation).
  - Phase 2 per (head, 512-query-block): scoresT[k, q] = kT-chunk.T @ qT,
    exp on ACT, gpsimd affine_select zeroes the causal staircase on the
    diagonal chunks; mask-injection matmul for the last pair of each block.
    softmax denominator split to balance PE vs DVE (DVE paces phase 2):
    small blocks (qsb 0,1) compute Z by the ebias-broadcast sumexp matmul
    + reciprocal_approx_fast on [128,512]; big blocks (qsb 2,3) use a
    per-half DVE FMA (scalar_tensor_tensor) accumulating
    zacc[k,q] += ebias[k]*e[k,q] in bf16 SBUF, then 4 single-column bf16
    matmuls reduce zacc over partitions into Z columns in PSUM, DVE
    reciprocal on [128,4], and 4 identity-moving bf16 matmuls broadcast
    1/Z across partitions into acc[:,512:1024] (staged through SBUF for
    the normalize multiply - one PSUM operand per instruction). Saves
    ~100k PE moving columns; a 3-deep software pipeline with deferred
    normalize stage-2 keeps the PE FIFO from blocking on DVE.
  - Phase 3: O[s, e] = attnT-chunks.T @ out_projT (partial over dv-slice).
  - First w/x DMA transfers split small so the first matmuls start early.
"""
import os
import sys
import types

if "/opt/trn_rl_repo" not in sys.path:
    sys.path.insert(0, "/opt/trn_rl_repo")

import numpy as np

B, S, D, H = 4, 2048, 2048, 16
HD = D // H          # 128 head dim
HPC = H // 2         # 8 heads per core
EV = HPC * HD        # 1024 dv-slice per core
NKC = S // 128       # 16 key chunks
NQB = S // 512       # 4 query blocks
NDC = D // 128       # 16 contraction chunks

_NC_CACHE = {}
LAST_EXEC_NS = None
LAST_PER_CORE_NS = None


def _install_ntff_hook():
    try:
        import antenv
        if "antenv.axon_hooks" in sys.modules:
            return
        mod = types.ModuleType("antenv.axon_hooks")
        state = {"hook": None}
        mod.set_axon_ntff_profile_hook = lambda h: state.__setitem__("hook", h)
        mod.get_axon_ntff_profile_hook = lambda: state["hook"]
        sys.modules["antenv.axon_hooks"] = mod
        antenv.axon_hooks = mod
        from trn_agent_boot.trn_boot import _ntff_profile_via_ctypes
        mod.set_axon_ntff_profile_hook(
            _ntff_profile_via_ctypes("/opt/axon/libaxon_pjrt.so"))
    except Exception:
        pass


def _build_nc():
    import concourse.bacc as bacc
    import concourse.mybir as mybir
    import concourse.tile as tile

    F32 = mybir.dt.float32
    BF16 = mybir.dt.bfloat16
    EXP = mybir.ActivationFunctionType.Exp
    MULT = mybir.AluOpType.mult
    ADD = mybir.AluOpType.add

    nc = bacc.Bacc()
    # host-packed layouts (see _prepare_core_inputs):
    #   xt[dc*128+d, s]            = x[b][s, dc*128+d]
    #   wqk[ec*128+d, dc*128+e]    = Wqk[ec*128+e, dc*128+d]  (q rows scaled)
    #   wv[d, dc*1024+ev]          = Wv[ev, dc*128+d]
    #   pt[dv, dvc*2048+e]         = out_proj_w[e, g*EV + dvc*128+dv]
    #   ebias_f[i, h*16+kc]        = exp(attn_bias[g*8+h, kc*128+i])
    xt = nc.dram_tensor("xt", [D, S], BF16, kind="ExternalInput")
    wqk = nc.dram_tensor("wqk", [2 * EV, D], BF16, kind="ExternalInput")
    wv = nc.dram_tensor("wv", [128, NDC * EV], BF16, kind="ExternalInput")
    pt = nc.dram_tensor("pt", [128, HPC * D], BF16, kind="ExternalInput")
    ebias_f = nc.dram_tensor("ebias_f", [128, HPC * NKC], F32,
                             kind="ExternalInput")
    ebias_r = nc.dram_tensor("ebias_r", [128, HPC * NKC], BF16,
                             kind="ExternalInput")
    # additive causal mask: cmask[i, c] = 0 if c >= i else -1e38
    cmask = nc.dram_tensor("cmask", [128, 512], BF16, kind="ExternalInput")
    # 128x128 identity (stationary for mask-injection matmuls; moving for
    # the 1/Z partition-broadcast matmuls)
    idm = nc.dram_tensor("idm", [128, 128], BF16, kind="ExternalInput")
    ones = nc.dram_tensor("ones", [128, 1], BF16, kind="ExternalInput")
    out = nc.dram_tensor("o", [S, D], F32, kind="ExternalOutput")

    with tile.TileContext(nc) as tc:
        with tc.tile_pool(name="consts", bufs=1) as cp:
            ebf_t = cp.tile([128, HPC * NKC], F32, tag="ebf")
            ebr_t = cp.tile([128, HPC * NKC], BF16, tag="ebr")
            cm_t = cp.tile([128, 512], BF16, tag="cm")
            id_t = cp.tile([128, 128], BF16, tag="idm")
            ones_t = cp.tile([128, 1], BF16, tag="ones")

            with tc.tile_pool(name="mp", bufs=1) as mp, \
                    tc.tile_pool(name="pp", bufs=2, space="PSUM") as pp:
                # persistent across phases
                qk_sb = []
                for ec in range(16):
                    t = mp.tile([128, S], BF16, tag=f"qk{ec}", name=f"qk{ec}")
                    qk_sb.append(t)
                v_sb = []
                for s16 in range(NKC):
                    t = mp.tile([128, EV], BF16, tag=f"v{s16}", name=f"v{s16}")
                    v_sb.append(t)

                # ---------------- phase 1: projections ----------------
                with (
                    tc.tile_pool(name="xp", bufs=1) as xp,
                    tc.tile_pool(name="wvp", bufs=1) as wvp,
                    tc.tile_pool(name="wqkp", bufs=3) as wp,
                ):
                    # DMA issue order sets the critical path: the first 1a
                    # matmul needs only w[0:128, 0:128] + x[0:128, 0:512] —
                    # issue those as small leading range-transfers into the
                    # same tiles (deps are view-range granular), then the
                    # bulk tails, then the rest.
                    w0 = wp.tile([128, D], BF16, tag="w", name="w0")
                    nc.sync.dma_start(w0[:, 0:128], wqk[0:128, 0:128])
                    x0 = xp.tile([128, S], BF16, tag="x0", name="x0")
                    nc.sync.dma_start(x0[:, 0:512], xt[0:128, 0:512])
                    nc.sync.dma_start(x0[:, 512:1024], xt[0:128, 512:1024])
                    nc.sync.dma_start(x0[:, 1024:S], xt[0:128, 1024:S])
                    nc.sync.dma_start(w0[:, 128:512], wqk[0:128, 128:512])
                    nc.sync.dma_start(w0[:, 512:D], wqk[0:128, 512:D])
                    wq_fifo = [w0]
                    x_t = [x0]
                    for dc in range(1, 5):
                        t = xp.tile([128, S], BF16, tag=f"x{dc}",
                                    name=f"x{dc}")
                        nc.sync.dma_start(t[:], xt[128 * dc:128 * (dc + 1), :])
                        x_t.append(t)
                    nc.sync.dma_start(ebf_t[:], ebias_f[:])
                    nc.sync.dma_start(ebr_t[:], ebias_r[:])
                    nc.sync.dma_start(cm_t[:], cmask[:])
                    nc.sync.dma_start(id_t[:], idm[:])
                    nc.sync.dma_start(ones_t[:], ones[:])
                    for ec in range(1, 3):
                        t = wp.tile([128, D], BF16, tag="w")
                        nc.sync.dma_start(
                            t[:], wqk[128 * ec:128 * (ec + 1), :])
                        wq_fifo.append(t)
                    for dc in range(5, NDC):
                        t = xp.tile([128, S], BF16, tag=f"x{dc}",
                                    name=f"x{dc}")
                        nc.sync.dma_start(t[:], xt[128 * dc:128 * (dc + 1), :])
                        x_t.append(t)
                    wv_t = wvp.tile([128, NDC * EV], BF16, tag="wv")
                    nc.sync.dma_start(wv_t[:], wv[:])

                    def w_stat(ec, w_t, dc):
                        return w_t[:, 128 * dc:128 * (dc + 1)]

                    def x_mov(dc, sc):
                        return x_t[dc][:, 512 * sc:512 * (sc + 1)]

                    def x_mov128(dc, s16):
                        return x_t[dc][:, 128 * s16:128 * (s16 + 1)]

                    # 1a: qkT (stationary W block reused over 4 s-blocks)
                    for ec in range(16):
                        if ec + 3 < 16:
                            t = wp.tile([128, D], BF16, tag="w")
                            nc.sync.dma_start(
                                t[:],
                                wqk[128 * (ec + 3):128 * (ec + 4), :])
                            wq_fifo.append(t)
                        w_t = wq_fifo.pop(0)
                        psA = pp.tile([128, 1024], F32, tag="sc",
                                      name=f"qkA{ec}")
                        psB = pp.tile([128, 1024], F32, tag="acc",
                                      name=f"qkB{ec}")
                        for dc in range(NDC):
                            st = w_stat(ec, w_t, dc)
                            for sc in range(NQB):
                                dst = psA if sc < 2 else psB
                                nc.tensor.matmul(
                                    dst[:, 512 * (sc % 2):
                                        512 * (sc % 2 + 1)],
                                    st, x_mov(dc, sc),
                                    start=(dc == 0), stop=(dc == NDC - 1))
                        # evacuate: split across engines (ACT + DVE)
                        nc.scalar.copy(qk_sb[ec][:, 0:1024], psA[:])
                        nc.vector.tensor_copy(qk_sb[ec][:, 1024:2048],
                                              psB[:])

                    # 1b: v (stationary x block reused over 2 ev-halves);
                    # evacuation folds exp(alibi) per (head, key chunk).
                    for s16 in range(NKC):
                        ps = pp.tile([128, EV], F32,
                                     tag=("sc" if s16 % 2 == 0 else "acc"),
                                     name=f"v_ps{s16}")
                        for dc in range(NDC):
                            for evh in range(2):
                                nc.tensor.matmul(
                                    ps[:, 512 * evh:512 * (evh + 1)],
                                    x_mov128(dc, s16),
                                    wv_t[:, EV * dc + 512 * evh:
                                         EV * dc + 512 * (evh + 1)],
                                    start=(dc == 0), stop=(dc == NDC - 1))
                        for h in range(HPC):
                            col = h * NKC + s16
                            if s16 == NKC - 1 and h % 2 == 0:
                                nc.scalar.activation(
                                    v_sb[s16][:, 128 * h:128 * (h + 1)],
                                    ps[:, 128 * h:128 * (h + 1)],
                                    mybir.ActivationFunctionType.Identity,
                                    bias=0.0,
                                    scale=ebf_t[:, col:col + 1])
                            else:
                                nc.vector.tensor_scalar(
                                    out=v_sb[s16][:, 128 * h:
                                                  128 * (h + 1)],
                                    in0=ps[:, 128 * h:128 * (h + 1)],
                                    scalar1=ebf_t[:, col:col + 1],
                                    scalar2=None, op0=MULT)
                # x freed here

                with (
                    tc.tile_pool(name="attn", bufs=1) as ap,
                    tc.tile_pool(name="ptp", bufs=1) as ptp,
                ):
                    # per-(qsb, head) tiles so phase 3 reads don't falsely
                    # depend on late phase-2 writes
                    attn_sb = [[ap.tile([128, 512], BF16, tag=f"a{q}_{h}",
                                        name=f"a{q}_{h}")
                                for h in range(HPC)] for q in range(NQB)]
                    pt_t = ptp.tile([128, HPC * D], BF16, tag="pt")
                    nc.sync.dma_start(pt_t[:], pt[:])

                    # ---------------- phase 2: attention ----------------
                    # flat software pipeline over (head, qsb, chunk-pair):
                    # the attn-V matmuls of pair i are emitted after the
                    # scores+exp of pair i+1, across block boundaries, so
                    # the PE never waits on ACT at a boundary.  Block-end
                    # normalize is further split in two stages (Z-extract,
                    # then bcast+mul one slot later) so the PE FIFO never
                    # heads-of-line blocks on the DVE reciprocal.
                    with (
                        tc.tile_pool(name="expt", bufs=6) as ep,
                        tc.tile_pool(name="zp", bufs=3) as zp,
                        tc.tile_pool(name="rp", bufs=3) as rp,
                    ):
                        tasks = []
                        for h in range(HPC):
                            for qsb in range(NQB):
                                for kp in range(2 * qsb + 2):
                                    tasks.append((h, qsb, kp))

                        state = {}    # (h, qsb) -> (acc, zacc) tiles
                        pending = []  # depth-2 software pipeline
                        post = []     # deferred normalize stage-2 closures

                        def emit_norm2(h, qsb, acc, r4b):
                            # stage 2: broadcast 1/Z across partitions and
                            # normalize.  (an instruction may read only ONE
                            # operand from PSUM, so stage the broadcast
                            # through SBUF before the multiply)
                            for i in range(4):
                                nc.tensor.matmul(
                                    acc[:, 512 + 128 * i:640 + 128 * i],
                                    r4b[:, i:i + 1].broadcast_to([128, 128]),
                                    id_t[:], start=True, stop=True)
                            rr_sb = rp.tile([128, 512], BF16, tag="rr")
                            nc.vector.tensor_copy(rr_sb[:],
                                                  acc[:, 512:1024])
                            nc.vector.tensor_mul(
                                attn_sb[qsb][h][:], acc[:, 0:512],
                                rr_sb[:])

                        def emit_se_at(task, e_t):
                            h, qsb, kp = task
                            nkc = 4 * qsb + 4
                            acc, zacc = state[(h, qsb)]
                            for half in range(2):
                                kc = 2 * kp + half
                                p = kc - 4 * qsb
                                off = 128 * p if p > 0 else 0
                                col = h * NKC + kc
                                e_sl = e_t[:, half, off:512]
                                if zacc is None:
                                    # hybrid: small blocks compute Z by the
                                    # ebias-broadcast sumexp matmul (cheap
                                    # here, relieves the DVE which paces
                                    # phase 2)
                                    nc.tensor.matmul(
                                        acc[:, 512 + off:1024],
                                        ebr_t[:, col:col + 1]
                                        .broadcast_to([128, 128]),
                                        e_sl,
                                        start=(kc == 0),
                                        stop=(kc == nkc - 1))
                                nc.tensor.matmul(
                                    acc[:, off:512],
                                    v_sb[kc][:, 128 * h:128 * (h + 1)],
                                    e_sl,
                                    start=(kc == 0), stop=(kc == nkc - 1))
                            if kp == nkc // 2 - 1:
                                if zacc is None:
                                    rrf = rp.tile([128, 512], F32,
                                                  tag="rrf")
                                    nc.vector.reciprocal_approx_fast(
                                        rrf[:], acc[:, 512:1024])
                                    nc.vector.tensor_mul(
                                        attn_sb[qsb][h][:], acc[:, 0:512],
                                        rrf[:])
                                    del state[(h, qsb)]
                                    return
                                # block done: Z = colsum(zacc) via 4 single
                                # column matmuls, then reciprocal.
                                for i in range(4):
                                    nc.tensor.matmul(
                                        acc[:, 1020 + i:1021 + i],
                                        zacc[:, 128 * i:128 * (i + 1)],
                                        ones_t[:], start=True, stop=True)
                                r4f = rp.tile([128, 4], F32, tag="r4f")
                                r4b = rp.tile([128, 4], BF16, tag="r4b")
                                nc.vector.reciprocal(r4f[:],
                                                     acc[:, 1020:1024])
                                nc.vector.tensor_copy(r4b[:], r4f[:])
                                post.append((h, qsb, acc, r4b))
                                del state[(h, qsb)]

                        for task in tasks:
                            h, qsb, kp = task
                            if kp == 0:
                                state[(h, qsb)] = (
                                    pp.tile([128, 1024], F32, tag="acc",
                                            name=f"acc{h}_{qsb}"),
                                    zp.tile([128, 512], BF16, tag="z",
                                            name=f"z{h}_{qsb}")
                                    if qsb >= 2 else None,
                                )
                            acc, zacc = state[(h, qsb)]
                            qt = qk_sb[h]
                            kt = qk_sb[HPC + h]
                            q0 = 512 * qsb
                            sc_ps = pp.tile([128, 2, 512], F32, tag="sc",
                                            name=f"sc{h}_{qsb}_{kp}")
                            e_t = ep.tile([128, 2, 512], BF16, tag="e")
                            diag = (2 * kp + 1 >= 4 * qsb)
                            # mask-injection matmul (identity stationary +
                            # additive -1e38 mask, exp yields exact zeros,
                            # no gpsimd op) for the last pair of each block.
                            mask_mm = (diag and
                                       (kp == 2 * qsb + 1 or qsb <= 1))
                            for half in range(2):
                                kc = 2 * kp + half
                                p = kc - 4 * qsb
                                off = 128 * p if p > 0 else 0
                                if mask_mm:
                                    nc.tensor.matmul(
                                        sc_ps[:, half, off:512],
                                        id_t[:],
                                        cm_t[:, 0:512 - off],
                                        start=True, stop=False)
                                nc.tensor.matmul(
                                    sc_ps[:, half, off:512],
                                    kt[:, 128 * kc:128 * (kc + 1)],
                                    qt[:, q0 + off:q0 + 512],
                                    start=not mask_mm, stop=True)
                            if not diag:
                                nc.scalar.activation(
                                    e_t[:], sc_ps[:], EXP,
                                    bias=0.0, scale=1.0)
                            elif mask_mm:
                                for half in range(2):
                                    off = 128 * (2 * kp + half - 4 * qsb)
                                    off = max(off, 0)
                                    nc.scalar.activation(
                                        e_t[:, half, off:512],
                                        sc_ps[:, half, off:512],
                                        EXP, bias=0.0, scale=1.0)
                            else:
                                # per-half exp then mask so each half's
                                # affine_select overlaps the other's exp
                                for half in range(2):
                                    kc = 2 * kp + half
                                    p = kc - 4 * qsb
                                    off = 128 * p if p > 0 else 0
                                    nc.scalar.activation(
                                        e_t[:, half, off:512],
                                        sc_ps[:, half, off:512],
                                        EXP, bias=0.0, scale=1.0)
                                    # keep local col c >= partition i
                                    nc.gpsimd.affine_select(
                                        out=e_t[:, half, off:512],
                                        in_=e_t[:, half, off:512],
                                        compare_op=mybir.AluOpType.is_ge,
                                        fill=0.0, base=0,
                                        pattern=[[1, 512 - off]],
                                        channel_multiplier=-1)
                            # softmax denominator accumulation on DVE:
                            # zacc[:, off:512] (+)= ebias * e_half
                            # (big blocks only; small blocks use the
                            # sumexp matmul in emit_se_at)
                            for half in range(2) if qsb >= 2 else []:
                                kc = 2 * kp + half
                                p = kc - 4 * qsb
                                off = 128 * p if p > 0 else 0
                                col = h * NKC + kc
                                e_sl = e_t[:, half, off:512]
                                if kc == 0:
                                    nc.vector.tensor_scalar(
                                        out=zacc[:, 0:512], in0=e_sl,
                                        scalar1=ebf_t[:, col:col + 1],
                                        scalar2=None, op0=MULT)
                                else:
                                    nc.vector.scalar_tensor_tensor(
                                        out=zacc[:, off:512], in0=e_sl,
                                        scalar=ebf_t[:, col:col + 1],
                                        in1=zacc[:, off:512],
                                        op0=MULT, op1=ADD)
                            if len(pending) == 3:
                                emit_se_at(*pending.pop(0))
                                if post:
                                    emit_norm2(*post.pop(0))
                            pending.append((task, e_t))
                        for p_ in pending:
                            emit_se_at(*p_)
                        for p_ in post:
                            emit_norm2(*p_)
                        post.clear()

                        # ------------- phase 3: out_proj partial ----------
                        # PSUM tiles reuse the same pool tags so the first
                        # out_proj matmul doesn't wait on a pool close
                        # barrier behind the last softmax normalize.
                        with tc.tile_pool(name="st3", bufs=2) as sp3:
                            for s16 in range(NKC):
                                qsb, sl = s16 // 4, s16 % 4
                                ps0 = pp.tile([128, 1024], F32, tag="sc",
                                              name=f"o3a_{s16}")
                                ps1 = pp.tile([128, 1024], F32, tag="acc",
                                              name=f"o3b_{s16}")
                                for dvc in range(HPC):
                                    a_sl = attn_sb[qsb][dvc][:, 128 * sl:
                                                             128 * (sl + 1)]
                                    for e2 in range(2):
                                        nc.tensor.matmul(
                                            ps0[:, 512 * e2:512 * (e2 + 1)],
                                            a_sl,
                                            pt_t[:, D * dvc + 512 * e2:
                                                 D * dvc + 512 * (e2 + 1)],
                                            start=(dvc == 0),
                                            stop=(dvc == HPC - 1))
                                    for e2 in range(2):
                                        nc.tensor.matmul(
                                            ps1[:, 512 * e2:512 * (e2 + 1)],
                                            a_sl,
                                            pt_t[:, D * dvc + 1024 +
                                                 512 * e2:
                                                 D * dvc + 1024 +
                                                 512 * (e2 + 1)],
                                            start=(dvc == 0),
                                            stop=(dvc == HPC - 1))
                                st = sp3.tile([128, D], F32, tag="st")
                                for e2 in range(2):
                                    nc.scalar.copy(
                                        st[:, 512 * e2:512 * (e2 + 1)],
                                        ps0[:, 512 * e2:512 * (e2 + 1)])
                                    nc.sync.dma_start(
                                        out[128 * s16:128 * (s16 + 1),
                                            512 * e2:512 * (e2 + 1)],
                                        st[:, 512 * e2:512 * (e2 + 1)])
                                for e2 in range(2):
                                    nc.scalar.copy(
                                        st[:, 1024 + 512 * e2:
                                           1024 + 512 * (e2 + 1)],
                                        ps1[:, 512 * e2:512 * (e2 + 1)])
                                    nc.sync.dma_start(
                                        out[128 * s16:128 * (s16 + 1),
                                            1024 + 512 * e2:
                                            1024 + 512 * (e2 + 1)],
                                        st[:, 1024 + 512 * e2:
                                           1024 + 512 * (e2 + 1)])
    nc.finalize()
    return nc


def _get_nc():
    if "nc" not in _NC_CACHE:
        _NC_CACHE["nc"] = _build_nc()
    return _NC_CACHE["nc"]


def _prepare_core_inputs(x, Wqkv_w, out_proj_w, attn_bias):
    import ml_dtypes
    BF = ml_dtypes.bfloat16
    scale = 1.0 / np.sqrt(HD)
    cmask = np.where(np.arange(512)[None, :] >= np.arange(128)[:, None],
                     np.float32(0.0), np.float32(-1e38)).astype(BF)
    idm = np.eye(128, dtype=np.float32).astype(BF)
    ones = np.ones((128, 1), dtype=np.float32)
    in_maps = []
    for c in range(8):
        b, g = c // 2, c % 2
        hlo, hhi = g * EV, (g + 1) * EV
        wq = Wqkv_w[hlo:hhi] * scale            # [1024, D]
        wk = Wqkv_w[D + hlo:D + hhi]            # [1024, D]
        wvm = Wqkv_w[2 * D + hlo:2 * D + hhi]   # [1024, D]
        # wqk packed: [ec, d, dc, e] <- Wqk[ec*128+e, dc*128+d]
        wqk_cat = np.concatenate([wq, wk], axis=0)        # [2048, D]
        wqk_p = np.ascontiguousarray(
            wqk_cat.reshape(16, 128, 16, 128).transpose(0, 3, 2, 1)
            .reshape(2 * EV, D)).astype(BF)
        # wv packed: [d, dc, ev] <- Wv[ev, dc*128+d]
        wv_p = np.ascontiguousarray(
            wvm.reshape(EV, 16, 128).transpose(2, 1, 0)
            .reshape(128, NDC * EV)).astype(BF)
        # pt packed: [dv, dvc, e] <- P[dvc*128+dv, e], P = out_proj[:, sl].T
        ptm = out_proj_w[:, hlo:hhi].T                     # [1024, D]
        pt_p = np.ascontiguousarray(
            ptm.reshape(HPC, 128, D).transpose(1, 0, 2)
            .reshape(128, HPC * D)).astype(BF)
        xt = np.ascontiguousarray(x[b].T).astype(BF)       # [D, S]
        # ebias[i, h*16+kc] = exp(attn_bias[g*8+h, kc*128+i])
        bias_g = attn_bias[0, g * HPC:(g + 1) * HPC, 0, :]  # [8, S]
        ebias = np.exp(np.ascontiguousarray(
            bias_g.reshape(HPC, NKC, 128).transpose(2, 0, 1)
            .reshape(128, HPC * NKC)).astype(np.float64)).astype(np.float32)
        in_maps.append({
            "xt": xt, "wqk": wqk_p, "wv": wv_p, "pt": pt_p,
            "ebias_f": ebias, "ebias_r": ebias.astype(BF),
            "cmask": cmask, "idm": idm, "ones": ones.astype(BF),
        })
    return in_maps


def kernel(x, Wqkv_w, out_proj_w, attn_bias, key_padding_mask=None):
    """Full inputs in, full [B, S, D] float32 output out.

    key_padding_mask is all-True for this problem spec and is ignored.
    """
    global LAST_EXEC_NS, LAST_PER_CORE_NS
    from concourse.bass_utils import run_bass_kernel_spmd

    x = np.asarray(x, dtype=np.float32)
    Wqkv_w = np.asarray(Wqkv_w, dtype=np.float32)
    out_proj_w = np.asarray(out_proj_w, dtype=np.float32)
    attn_bias = np.asarray(attn_bias, dtype=np.float32)

    trace = bool(int(os.environ.get("KERNEL_TRACE", "0")))
    if trace:
        _install_ntff_hook()

    nc = _get_nc()
    in_maps = _prepare_core_inputs(x, Wqkv_w, out_proj_w, attn_bias)
    kwargs = {}
    if trace:
        kwargs.update(trace=True, trace_cores=list(range(8)))
    res = run_bass_kernel_spmd(nc, in_maps, core_ids=list(range(8)), **kwargs)
    LAST_EXEC_NS = res.exec_time_ns
    LAST_PER_CORE_NS = res.mean_exec_time_ns

    out = np.empty((B, S, D), dtype=np.float32)
    for b in range(B):
        out[b] = res.results[2 * b]["o"] + res.results[2 * b + 1]["o"]
    return out
